# revision 5
# baseline (speedup 1.0000x reference)
"""BinaryTreeLSTM (easy-first / Gumbel TreeLSTM, eval-mode hard argmax) on 8 TRN2
NeuronCores.

Strategy (from sharding hint): data-parallel over batch. Each of the 8 cores
runs the full 63-step depth loop for its 8 sentences, entirely SBUF-resident:

  layout: feature-major. h/c live as [128 partitions (feature chunk), 4 chunks,
  64 cols] fp32 SBUF tiles, column = sentence*64 + position.

  per step i (Lc = 63-i pairs, m = 8*Lc):
    PE : v[n,:] = sum_k WcT[k,n-tile]^T @ concat(h_l, h_r)[k]   (20 n-tiles x 8 k)
    ACT: gates = sigmoid/tanh(v + bias) straight out of PSUM (bias pre-folded,
         +1.0 on both forget gates baked in host-side)
    DVE: c_new = cl*s_fl + cr*s_fr + tanh_u*s_i ; h_new = s_o*tanh(c_new)
    PE : scores = q . h_new  -> [1, (8, Lc)] PSUM
    DVE: per-sentence argmax -> one-hot sel row + right-shift mask row
         (iota compare; inactive sentences forced to keep-state)
    GPS: partition_broadcast mask rows to [128, m] uint8
    DVE: in-place column blend:  h[k*] <- h_new[k*]  then  h[l] <- h[l+1] (l>k*)

The argmax selection is numerically brutal: the min top-2 score gap over the
whole run is 4.7e-6, so composition matmuls run in true fp32 (PE 4-pass mode).
A flipped argmax rebuilds a different tree and fails the whole sentence.
"""

import numpy as np

import concourse.bass as bass
import concourse.tile as tile
from concourse import bacc, mybir
from concourse.bass_utils import run_bass_kernel_spmd

dt = mybir.dt
AF = mybir.ActivationFunctionType
ALU = mybir.AluOpType

B, L, W, H = 64, 64, 512, 512
NCORES = 8
BL = B // NCORES          # sentences per core
K2H = 2 * H               # 1024 contraction dim
N5H = 5 * H               # 2560 output dim
NK = K2H // 128           # 8 k-chunks
NN = N5H // 128           # 20 n-tiles
NF = H // 128             # 4 feature chunks

_cached = {}


def _build(precision="fp32"):
    nc = bacc.Bacc()

    inpT_d = nc.declare_dram_parameter("inpT", [W, BL * L], dt.float32, isOutput=False)
    wwT_d = nc.declare_dram_parameter("wwT", [W, K2H], dt.float32, isOutput=False)
    wcT_d = nc.declare_dram_parameter("wcT", [K2H, N5H], dt.float32, isOutput=False)
    bw_d = nc.declare_dram_parameter("bw_t", [128, K2H // 128], dt.float32, isOutput=False)
    bc_d = nc.declare_dram_parameter("bc_t", [128, NN], dt.float32, isOutput=False)
    q_d = nc.declare_dram_parameter("q_t", [128, NF], dt.float32, isOutput=False)
    act_d = nc.declare_dram_parameter("act_row", [1, 512], dt.float32, isOutput=False)
    iota_d = nc.declare_dram_parameter("iota_row", [1, 64], dt.float32, isOutput=False)
    outh_d = nc.declare_dram_parameter("out_h", [128, NF * BL], dt.float32, isOutput=True)
    outc_d = nc.declare_dram_parameter("out_c", [128, NF * BL], dt.float32, isOutput=True)

    with tile.TileContext(nc) as tc:
        with (
            tc.tile_pool(name="persist", bufs=1) as persist,
            tc.tile_pool(name="psum", bufs=1, space="PSUM") as psum,
        ):
            wc_t = persist.tile([128, NK, N5H], dt.float32)
            for k in range(NK):
                nc.sync.dma_start(
                    wc_t[:, k, :], wcT_d[:].rearrange("(k p) n -> p k n", p=128)[:, k, :]
                )
            bc_t = persist.tile([128, NN], dt.float32)
            nc.sync.dma_start(bc_t[:], bc_d[:])
            bw_t = persist.tile([128, K2H // 128], dt.float32)
            nc.sync.dma_start(bw_t[:], bw_d[:])
            q_t = persist.tile([128, NF], dt.float32)
            nc.sync.dma_start(q_t[:], q_d[:])
            act_t = persist.tile([1, 512], dt.float32)
            nc.sync.dma_start(act_t[:], act_d[:])
            iota_t = persist.tile([1, 64], dt.float32)
            nc.sync.dma_start(iota_t[:], iota_d[:])

            h_t = persist.tile([128, NF, BL * 64], dt.float32)
            c_t = persist.tile([128, NF, BL * 64], dt.float32)

            # ---------------- phase 0: word linear ----------------
            with tc.tile_pool(name="ph0", bufs=1) as ph0:
                ww_t = ph0.tile([128, 4, K2H], dt.float32)
                for k in range(4):
                    nc.sync.dma_start(
                        ww_t[:, k, :],
                        wwT_d[:].rearrange("(k p) n -> p k n", p=128)[:, k, :],
                    )
                ix_t = ph0.tile([128, 4, BL * L], dt.float32)
                for k in range(4):
                    nc.sync.dma_start(
                        ix_t[:, k, :],
                        inpT_d[:].rearrange("(k p) m -> p k m", p=128)[:, k, :],
                    )
                for n in range(K2H // 128):
                    p0 = psum.tile([128, BL * L], dt.float32, tag="v", bufs=6, name="p0")
                    for k in range(4):
                        nc.tensor.matmul(
                            p0[:],
                            ww_t[:, k, n * 128 : (n + 1) * 128],
                            ix_t[:, k, :],
                            start=(k == 0),
                            stop=(k == 3),
                        )
                    dst = h_t if n < NF else c_t
                    nc.scalar.activation(
                        dst[:, n % NF, :], p0[:], AF.Identity,
                        bias=bw_t[:, n : n + 1],
                    )

            # ---------------- 63 tree steps ----------------
            with (
                tc.tile_pool(name="gates", bufs=1) as gates,
                tc.tile_pool(name="temps", bufs=1) as temps,
                tc.tile_pool(name="rows", bufs=1) as rows,
                tc.tile_pool(name="masks", bufs=1) as masks,
            ):
                for i in range(L - 1):
                    Lc = L - 1 - i
                    m = BL * Lc

                    def hview(t, off):
                        # [128, NF, BL, Lc] column view at position offset `off`
                        return t[:].rearrange("p c (b l) -> p c b l", l=64)[
                            :, :, :, off : off + Lc
                        ]

                    # ---- composition matmuls
                    g_i = gates.tile([128, NF, m], dt.float32, tag="g0", name="g_i")
                    g_fl = gates.tile([128, NF, m], dt.float32, tag="g1", name="g_fl")
                    g_fr = gates.tile([128, NF, m], dt.float32, tag="g2", name="g_fr")
                    g_u = gates.tile([128, NF, m], dt.float32, tag="g3", name="g_u")
                    g_o = gates.tile([128, NF, m], dt.float32, tag="g4", name="g_o")
                    gtiles = [g_i, g_fl, g_fr, g_u, g_o]
                    gfuncs = [AF.Sigmoid, AF.Sigmoid, AF.Sigmoid, AF.Tanh, AF.Sigmoid]

                    for f in range(NF):
                        for g in range(5):
                            n = g * NF + f
                            vt = psum.tile([128, m], dt.float32, tag="v", bufs=6, name="vt")
                            for k in range(NK):
                                off = k * 512 if k < NF else (k - NF) * 512 + 1
                                rhs = h_t[:, k % NF, :].rearrange(
                                    "p (b l) -> p b l", l=64
                                )[:, :, (0 if k < NF else 1) :][:, :, :Lc]
                                nc.tensor.matmul(
                                    vt[:],
                                    wc_t[:, k, n * 128 : (n + 1) * 128],
                                    rhs,
                                    start=(k == 0),
                                    stop=(k == NK - 1),
                                )
                            nc.scalar.activation(
                                gtiles[g][:, f, :], vt[:], gfuncs[g],
                                bias=bc_t[:, n : n + 1],
                            )

                    # ---- cell combine (hn/cn live in the same b*64+l layout
                    # as h/c so copy_predicated operand views collapse alike)
                    cl = hview(c_t, 0)
                    cr = hview(c_t, 1)
                    m1 = temps.tile([128, NF, m], dt.float32, tag="m1", name="m1")
                    m2 = temps.tile([128, NF, m], dt.float32, tag="m2", name="m2")
                    m3 = temps.tile([128, NF, m], dt.float32, tag="m3", name="m3")
                    cn_t = temps.tile([128, NF, 512], dt.float32, tag="cn", name="cn_t")
                    hn_t = temps.tile([128, NF, 512], dt.float32, tag="hn", name="hn_t")
                    tc_t = temps.tile([128, NF, m], dt.float32, tag="tc", name="tc_t")
                    cn_v = hview(cn_t, 0)
                    hn_v = hview(hn_t, 0)
                    nc.vector.tensor_mul(m1[:], g_fl[:], cl)
                    nc.vector.tensor_mul(m2[:], g_fr[:], cr)
                    nc.gpsimd.tensor_tensor(m3[:], g_u[:], g_i[:], op=ALU.mult)
                    nc.vector.tensor_add(m1[:], m1[:], m2[:])
                    nc.vector.tensor_add(cn_v, m1[:], m3[:])
                    nc.scalar.activation(tc_t[:], cn_v, AF.Tanh)
                    nc.vector.tensor_mul(hn_v, g_o[:], tc_t[:])

                    # ---- selection masks
                    sel_u8 = masks.tile([1, m], dt.uint8, tag="selu8", name="sel_u8")
                    if i < L - 2:
                        ps_s = psum.tile([1, m], dt.float32, tag="s", bufs=2, name="ps_s")
                        for f in range(NF):
                            nc.tensor.matmul(
                                ps_s[:],
                                q_t[:, f : f + 1],
                                hn_t[:, f, :].rearrange("p (b l) -> p b l", l=64)[
                                    :, :, :Lc
                                ],
                                start=(f == 0),
                                stop=(f == NF - 1),
                            )
                        ps_v = ps_s[:].rearrange("p (b l) -> p b l", b=BL)
                        mx = rows.tile([1, BL], dt.float32, tag="mx", name="mx")
                        nc.vector.tensor_reduce(
                            mx[:], ps_v, axis=mybir.AxisListType.X, op=ALU.max
                        )
                        selr = rows.tile([1, m], dt.float32, tag="selr", name="selr")
                        selr_v = selr[:].rearrange("p (b l) -> p b l", b=BL)
                        nc.vector.tensor_tensor(
                            selr_v, ps_v,
                            mx[:].unsqueeze(2).broadcast_to((1, BL, Lc)),
                            op=ALU.is_equal,
                        )
                        iota_b = iota_t[0:1, 0:Lc].unsqueeze(1).broadcast_to((1, BL, Lc))
                        sxi = rows.tile([1, m], dt.float32, tag="sxi", name="sxi")
                        sxi_v = sxi[:].rearrange("p (b l) -> p b l", b=BL)
                        nc.vector.tensor_tensor(sxi_v, selr_v, iota_b, op=ALU.mult)
                        kidx = rows.tile([1, BL], dt.float32, tag="kidx", name="kidx")
                        nc.vector.tensor_reduce(
                            kidx[:], sxi_v, axis=mybir.AxisListType.X, op=ALU.add
                        )
                        act_b = (
                            act_t[0:1, i * BL : (i + 1) * BL]
                            .unsqueeze(2)
                            .broadcast_to((1, BL, Lc))
                        )
                        sel_v8 = sel_u8[:].rearrange("p (b l) -> p b l", b=BL)
                        nc.vector.tensor_tensor(sel_v8, selr_v, act_b, op=ALU.mult)
                        if Lc > 1:
                            rgt = rows.tile([1, m], dt.float32, tag="rgt", name="rgt")
                            rgt_v = rgt[:].rearrange("p (b l) -> p b l", b=BL)
                            nc.vector.tensor_tensor(
                                rgt_v, iota_b,
                                kidx[:].unsqueeze(2).broadcast_to((1, BL, Lc)),
                                op=ALU.is_gt,
                            )
                            rm_u8 = masks.tile([1, m], dt.uint8, tag="rmu8", name="rm_u8")
                            rm_v8 = rm_u8[:].rearrange("p (b l) -> p b l", b=BL)
                            nc.vector.tensor_tensor(rm_v8, rgt_v, act_b, op=ALU.mult)
                    else:
                        # last step: select is skipped; "sel" = active mask
                        nc.vector.tensor_copy(
                            sel_u8[:], act_t[0:1, i * BL : (i + 1) * BL]
                        )

                    # ---- broadcast masks (to b*64+l layout) and blend in place
                    sel128 = masks.tile([128, m], dt.uint8, tag="sel128", name="sel128")
                    nc.gpsimd.partition_broadcast(sel128[:], sel_u8[:])
                    sel512 = masks.tile([128, NF, 512], dt.uint8, tag="sel512", name="sel512")
                    nc.vector.tensor_copy(
                        hview(sel512, 0),
                        sel128[:]
                        .rearrange("p (b l) -> p b l", b=BL)
                        .unsqueeze(1)
                        .broadcast_to((128, NF, BL, Lc)),
                    )
                    nc.vector.copy_predicated(hview(h_t, 0), hview(sel512, 0), hn_v)
                    nc.vector.copy_predicated(hview(c_t, 0), hview(sel512, 0), cn_v)
                    if i < L - 2 and Lc > 1:
                        rm128 = masks.tile([128, m], dt.uint8, tag="rm128", name="rm128")
                        nc.gpsimd.partition_broadcast(rm128[:], rm_u8[:])
                        rm512 = masks.tile([128, NF, 512], dt.uint8, tag="rm512", name="rm512")
                        nc.vector.tensor_copy(
                            hview(rm512, 0),
                            rm128[:]
                            .rearrange("p (b l) -> p b l", b=BL)
                            .unsqueeze(1)
                            .broadcast_to((128, NF, BL, Lc)),
                        )
                        nc.vector.copy_predicated(hview(h_t, 0), hview(rm512, 0), hview(h_t, 1))
                        nc.vector.copy_predicated(hview(c_t, 0), hview(rm512, 0), hview(c_t, 1))

            # ---------------- output ----------------
            hroot = h_t[:].rearrange("p c (b l) -> p c b l", l=64)[:, :, :, 0]
            croot = c_t[:].rearrange("p c (b l) -> p c b l", l=64)[:, :, :, 0]
            nc.sync.dma_start(outh_d[:].rearrange("p (c b) -> p c b", b=BL), hroot)
            nc.sync.dma_start(outc_d[:].rearrange("p (c b) -> p c b", b=BL), croot)

    nc.compile()
    return nc


def _prep_inputs(inputs):
    inp = np.ascontiguousarray(np.asarray(inputs["inp"], dtype=np.float32))
    length = np.asarray(inputs["length"]).astype(np.int64)
    Ww = np.asarray(inputs["Ww"], dtype=np.float32)
    bw = np.asarray(inputs["bw"], dtype=np.float32)
    Wc = np.asarray(inputs["Wc"], dtype=np.float32)
    bc = np.asarray(inputs["bc"], dtype=np.float32)
    q = np.asarray(inputs["q"], dtype=np.float32)

    wwT = np.ascontiguousarray(Ww.T)                      # [512, 1024]
    wcT = np.ascontiguousarray(Wc.T)                      # [1024, 2560]
    bw_t = np.ascontiguousarray(bw.reshape(K2H // 128, 128).T)
    bc_adj = bc.copy()
    bc_adj[H : 3 * H] += 1.0                              # +1 on both forget gates
    bc_t = np.ascontiguousarray(bc_adj.reshape(NN, 128).T)
    q_t = np.ascontiguousarray(q.reshape(NF, 128).T)
    iota_row = np.arange(64, dtype=np.float32).reshape(1, 64)

    in_maps = []
    for c in range(NCORES):
        sl = slice(c * BL, (c + 1) * BL)
        inpT = np.ascontiguousarray(inp[sl].reshape(BL * L, W).T)  # [512, 512]
        lc = length[sl]
        act = np.zeros((1, 512), np.float32)
        for i in range(L - 1):
            act[0, i * BL : (i + 1) * BL] = (lc > i).astype(np.float32)
        in_maps.append(
            {
                "inpT": inpT,
                "wwT": wwT,
                "wcT": wcT,
                "bw_t": bw_t,
                "bc_t": bc_t,
                "q_t": q_t,
                "act_row": act,
                "iota_row": iota_row,
            }
        )
    return in_maps


def _postprocess(results):
    hs, cs = [], []
    for r in results:
        oh = r["out_h"].reshape(128, NF, BL).transpose(2, 1, 0).reshape(BL, H)
        oc = r["out_c"].reshape(128, NF, BL).transpose(2, 1, 0).reshape(BL, H)
        hs.append(oh)
        cs.append(oc)
    return np.concatenate(hs, 0), np.concatenate(cs, 0)


def kernel(**inputs):
    if "nc" not in _cached:
        _cached["nc"] = _build()
    nc = _cached["nc"]
    in_maps = _prep_inputs(inputs)
    res = run_bass_kernel_spmd(nc, in_maps, list(range(NCORES)))
    return _postprocess(res.results)


def kernel_profiled(**inputs):
    """test.py helper: also returns BassKernelResults for timing/trace."""
    if "nc" not in _cached:
        _cached["nc"] = _build()
    nc = _cached["nc"]
    in_maps = _prep_inputs(inputs)
    try:
        res = run_bass_kernel_spmd(nc, in_maps, list(range(NCORES)), trace=True)
    except Exception as e:
        print("trace failed, running untraced:", e)
        res = run_bass_kernel_spmd(nc, in_maps, list(range(NCORES)))
    return _postprocess(res.results), res


# revision 6
# speedup vs baseline: 1.6234x; 1.6234x over previous
"""BinaryTreeLSTM (easy-first / Gumbel TreeLSTM, eval-mode hard argmax) on 8 TRN2
NeuronCores.

Strategy (sharding hint): data-parallel over batch. Each core runs the full
63-step depth loop for its 8 sentences, entirely SBUF-resident, feature-major
(h/c as [128 part, 4 chunks, 512 cols] tiles, column = sentence*64 + position).

Numerics: the argmax selection is brutal — min top-2 score gap over the run is
4.7e-6, and one flipped argmax rebuilds a different tree for that sentence
(absmax error ~0.8). bf16 matmuls flip 370 times; fp32(4cy/row) and an
fp16 hi/lo split (3 passes at 1cy/row, error ~1e-7) both flip zero times
(verified offline against the fixed key-0 inputs). Default: fp16x2 — the
recurrent h state is kept permanently split as (hh, hl) fp16 pairs; the column
blend is pure copies so the split survives exactly; c stays fp32 (never enters
a matmul).

Per step i (Lc = 63-i pairs, m = 8*Lc):
  PE : v[n] = sum_k [Wch_k^T hh_k + Wch_k^T hl_k + Wcl_k^T hh_k]  (20 n x 8 k x 3)
  ACT: 20 gate tiles sigmoid/tanh straight out of PSUM (bias pre-folded)
  DVE: per-chunk c_new/h_new combine (chunk-pipelined under the matmuls)
  PE : scores = q . h_new -> [1,(8,Lc)] PSUM, then K=1 ones-outer-product
       broadcast of the score row to 128 partitions (keeps masks full-width)
  DVE: per-sentence argmax -> one-hot sel mask + right-shift mask, uint8,
       written in the b*64+l column layout
  DVE: in-place blend per chunk: st[k*] <- new[k*]; st[l] <- st[l+1] (l>k*)
       for st in {hh, hl, c}
"""

import numpy as np

import concourse.bass as bass
import concourse.tile as tile
from concourse import bacc, mybir
from concourse.bass_utils import run_bass_kernel_spmd

dt = mybir.dt
AF = mybir.ActivationFunctionType
ALU = mybir.AluOpType

B, L, W, H = 64, 64, 512, 512
NCORES = 8
BL = B // NCORES          # sentences per core
K2H = 2 * H               # 1024 contraction dim
N5H = 5 * H               # 2560 output dim
NK = K2H // 128           # 8 k-chunks
NN = N5H // 128           # 20 n-tiles
NF = H // 128             # 4 feature chunks

PRECISION = "fp16x2"      # "fp32" fallback

_cached = {}


def _build(precision=PRECISION):
    nc = bacc.Bacc()
    f32 = dt.float32
    f16 = dt.float16

    inpT_d = nc.declare_dram_parameter("inpT", [W, BL * L], f32, isOutput=False)
    wwT_d = nc.declare_dram_parameter("wwT", [W, K2H], f32, isOutput=False)
    if precision == "fp16x2":
        wcTh_d = nc.declare_dram_parameter("wcTh", [K2H, N5H], f16, isOutput=False)
        wcTl_d = nc.declare_dram_parameter("wcTl", [K2H, N5H], f16, isOutput=False)
    else:
        wcT_d = nc.declare_dram_parameter("wcT", [K2H, N5H], f32, isOutput=False)
    bw_d = nc.declare_dram_parameter("bw_t", [128, NK], f32, isOutput=False)
    bc_d = nc.declare_dram_parameter("bc_t", [128, NN], f32, isOutput=False)
    q_d = nc.declare_dram_parameter("q_t", [128, NF], f32, isOutput=False)
    act_d = nc.declare_dram_parameter("act_row", [1, 512], f32, isOutput=False)
    iota_d = nc.declare_dram_parameter("iota_row", [1, 64], f32, isOutput=False)
    ones_d = nc.declare_dram_parameter("ones_row", [1, 128], f32, isOutput=False)
    outh_d = nc.declare_dram_parameter("out_h", [128, NF * BL], f32, isOutput=True)
    outc_d = nc.declare_dram_parameter("out_c", [128, NF * BL], f32, isOutput=True)

    def col_view(t, off, Lc):
        # [128, NF, BL, Lc] view of a [128, NF, 512] tile at position offset
        return t[:].rearrange("p c (b l) -> p c b l", l=64)[:, :, :, off : off + Lc]

    def chunk_view(t, f, off, Lc):
        # [128, BL, Lc] view of chunk f of a [128, NF, 512] tile
        return t[:, f, :].rearrange("p (b l) -> p b l", l=64)[:, :, off : off + Lc]

    def row_view(t, off, Lc):
        # [128, BL, Lc] view of a [128, 512] tile
        return t[:].rearrange("p (b l) -> p b l", l=64)[:, :, off : off + Lc]

    with tile.TileContext(nc) as tc:
        with (
            tc.tile_pool(name="persist", bufs=1) as persist,
            tc.tile_pool(name="psum", bufs=1, space="PSUM") as psum,
        ):
            if precision == "fp16x2":
                wch_t = persist.tile([128, NK, N5H], f16)
                wcl_t = persist.tile([128, NK, N5H], f16)
                for k in range(NK):
                    nc.sync.dma_start(
                        wch_t[:, k, :],
                        wcTh_d[:].rearrange("(k p) n -> p k n", p=128)[:, k, :],
                    )
                    nc.sync.dma_start(
                        wcl_t[:, k, :],
                        wcTl_d[:].rearrange("(k p) n -> p k n", p=128)[:, k, :],
                    )
            else:
                wc_t = persist.tile([128, NK, N5H], f32)
                for k in range(NK):
                    nc.sync.dma_start(
                        wc_t[:, k, :],
                        wcT_d[:].rearrange("(k p) n -> p k n", p=128)[:, k, :],
                    )
            bc_t = persist.tile([128, NN], f32)
            nc.sync.dma_start(bc_t[:], bc_d[:])
            bw_t = persist.tile([128, NK], f32)
            nc.sync.dma_start(bw_t[:], bw_d[:])
            q_t = persist.tile([128, NF], f32)
            nc.sync.dma_start(q_t[:], q_d[:])
            act_t = persist.tile([1, 512], f32)
            nc.sync.dma_start(act_t[:], act_d[:])
            iota_t = persist.tile([1, 64], f32)
            nc.sync.dma_start(iota_t[:], iota_d[:])
            ones_t = persist.tile([1, 128], f32)
            nc.sync.dma_start(ones_t[:], ones_d[:])

            act128 = persist.tile([128, 512], f32)
            nc.gpsimd.partition_broadcast(act128[:], act_t[:])
            iota128 = persist.tile([128, 64], f32)
            nc.gpsimd.partition_broadcast(iota128[:], iota_t[:])

            # recurrent state
            if precision == "fp16x2":
                hh_t = persist.tile([128, NF, 512], f16)
                hl_t = persist.tile([128, NF, 512], f16)
                hstate = [hh_t, hl_t]
            else:
                h_t = persist.tile([128, NF, 512], f32)
                hstate = [h_t]
            c_t = persist.tile([128, NF, 512], f32)

            # ---------------- phase 0: word linear (fp32) ----------------
            with tc.tile_pool(name="ph0", bufs=1) as ph0:
                ww_t = ph0.tile([128, 4, K2H], f32)
                for k in range(4):
                    nc.sync.dma_start(
                        ww_t[:, k, :],
                        wwT_d[:].rearrange("(k p) n -> p k n", p=128)[:, k, :],
                    )
                ix_t = ph0.tile([128, 4, BL * L], f32)
                for k in range(4):
                    nc.sync.dma_start(
                        ix_t[:, k, :],
                        inpT_d[:].rearrange("(k p) m -> p k m", p=128)[:, k, :],
                    )
                for n in range(NK):
                    p0 = psum.tile([128, BL * L], f32, tag="v", bufs=6, name="p0")
                    for k in range(4):
                        nc.tensor.matmul(
                            p0[:],
                            ww_t[:, k, n * 128 : (n + 1) * 128],
                            ix_t[:, k, :],
                            start=(k == 0),
                            stop=(k == 3),
                        )
                    if n < NF:
                        if precision == "fp16x2":
                            nc.scalar.activation(
                                hh_t[:, n, :], p0[:], AF.Identity,
                                bias=bw_t[:, n : n + 1],
                            )
                            nc.vector.scalar_tensor_tensor(
                                hl_t[:, n, :], p0[:], bw_t[:, n : n + 1],
                                hh_t[:, n, :], op0=ALU.add, op1=ALU.subtract,
                            )
                        else:
                            nc.scalar.activation(
                                h_t[:, n, :], p0[:], AF.Identity,
                                bias=bw_t[:, n : n + 1],
                            )
                    else:
                        nc.scalar.activation(
                            c_t[:, n - NF, :], p0[:], AF.Identity,
                            bias=bw_t[:, n : n + 1],
                        )

            # ---------------- 63 tree steps ----------------
            with (
                tc.tile_pool(name="gates", bufs=1) as gates,
                tc.tile_pool(name="temps", bufs=1) as temps,
                tc.tile_pool(name="rows", bufs=1) as rows,
                tc.tile_pool(name="masks", bufs=1) as masks,
            ):
                for i in range(L - 1):
                    Lc = L - 1 - i
                    m = BL * Lc

                    g_i = gates.tile([128, NF, m], f32, tag="g0", name="g_i")
                    g_fl = gates.tile([128, NF, m], f32, tag="g1", name="g_fl")
                    g_fr = gates.tile([128, NF, m], f32, tag="g2", name="g_fr")
                    g_u = gates.tile([128, NF, m], f32, tag="g3", name="g_u")
                    g_o = gates.tile([128, NF, m], f32, tag="g4", name="g_o")
                    gtiles = [g_i, g_fl, g_fr, g_u, g_o]
                    gfuncs = [AF.Sigmoid, AF.Sigmoid, AF.Sigmoid, AF.Tanh, AF.Sigmoid]

                    m1 = temps.tile([128, NF, m], f32, tag="m1", name="m1")
                    m2 = temps.tile([128, NF, m], f32, tag="m2", name="m2")
                    m3 = temps.tile([128, NF, m], f32, tag="m3", name="m3")
                    cn_t = temps.tile([128, NF, 512], f32, tag="cn", name="cn_t")
                    hn_t = temps.tile([128, NF, 512], f32, tag="hn", name="hn_t")
                    if precision == "fp16x2":
                        hnh_t = temps.tile([128, NF, 512], f16, tag="hnh", name="hnh_t")
                        hnl_t = temps.tile([128, NF, 512], f16, tag="hnl", name="hnl_t")
                        hnew = [hnh_t, hnl_t]
                    else:
                        hnew = [hn_t]

                    if i < L - 2:
                        ps_s = psum.tile([1, m], f32, tag="s", bufs=1, name="ps_s")

                    for f in range(NF):
                        # ---- composition matmuls for the 5 gates of chunk f
                        for g in range(5):
                            n = g * NF + f
                            vt = psum.tile([128, m], f32, tag="v", bufs=6, name="vt")
                            for k in range(NK):
                                fo = k % NF
                                off = 0 if k < NF else 1
                                if precision == "fp16x2":
                                    xh = chunk_view(hh_t, fo, off, Lc)
                                    xl = chunk_view(hl_t, fo, off, Lc)
                                    wh = wch_t[:, k, n * 128 : (n + 1) * 128]
                                    wl = wcl_t[:, k, n * 128 : (n + 1) * 128]
                                    nc.tensor.matmul(
                                        vt[:], wh, xh, start=(k == 0), stop=False
                                    )
                                    nc.tensor.matmul(
                                        vt[:], wh, xl, start=False, stop=False
                                    )
                                    nc.tensor.matmul(
                                        vt[:], wl, xh, start=False,
                                        stop=(k == NK - 1),
                                    )
                                else:
                                    nc.tensor.matmul(
                                        vt[:],
                                        wc_t[:, k, n * 128 : (n + 1) * 128],
                                        chunk_view(h_t, fo, off, Lc),
                                        start=(k == 0),
                                        stop=(k == NK - 1),
                                    )
                            nc.scalar.activation(
                                gtiles[g][:, f, :], vt[:], gfuncs[g],
                                bias=bc_t[:, n : n + 1],
                            )

                        # ---- combine for chunk f (overlaps later chunks' MMs)
                        cl_f = chunk_view(c_t, f, 0, Lc)
                        cr_f = chunk_view(c_t, f, 1, Lc)
                        cn_f = chunk_view(cn_t, f, 0, Lc)
                        hn_f = chunk_view(hn_t, f, 0, Lc)
                        nc.vector.tensor_mul(m1[:, f, :], g_fl[:, f, :], cl_f)
                        nc.vector.tensor_mul(m2[:, f, :], g_fr[:, f, :], cr_f)
                        nc.gpsimd.tensor_tensor(
                            m3[:, f, :], g_u[:, f, :], g_i[:, f, :], op=ALU.mult
                        )
                        nc.vector.tensor_add(m1[:, f, :], m1[:, f, :], m2[:, f, :])
                        nc.vector.tensor_add(cn_f, m1[:, f, :], m3[:, f, :])
                        nc.scalar.activation(m2[:, f, :], cn_f, AF.Tanh)
                        nc.vector.tensor_mul(hn_f, g_o[:, f, :], m2[:, f, :])
                        if i < L - 2:
                            nc.tensor.matmul(
                                ps_s[:],
                                q_t[:, f : f + 1],
                                hn_f,
                                start=(f == 0),
                                stop=(f == NF - 1),
                            )

                    if precision == "fp16x2":
                        # hi/lo split of h_new off the DVE critical path
                        nc.scalar.copy(col_view(hnh_t, 0, Lc), col_view(hn_t, 0, Lc))
                        nc.gpsimd.tensor_tensor(
                            col_view(hnl_t, 0, Lc),
                            col_view(hn_t, 0, Lc),
                            col_view(hnh_t, 0, Lc),
                            op=ALU.subtract,
                        )

                    # ---- selection masks (b*64+l layout, uint8)
                    sel_m = masks.tile([128, 512], dt.uint8, tag="selm", name="sel_m")
                    act_b = (
                        act128[:, i * BL : (i + 1) * BL]
                        .unsqueeze(2)
                        .broadcast_to((128, BL, Lc))
                    )
                    if i < L - 2:
                        srow = rows.tile([1, m], f32, tag="srow", name="srow")
                        nc.vector.tensor_copy(srow[:], ps_s[:])
                        ps_bc = psum.tile([128, m], f32, tag="bc", bufs=1, name="ps_bc")
                        nc.tensor.matmul(
                            ps_bc[:], ones_t[0:1, :], srow[:], start=True, stop=True
                        )
                        bc_v = ps_bc[:].rearrange("p (b l) -> p b l", b=BL)
                        mx = rows.tile([128, BL], f32, tag="mx", name="mx")
                        nc.vector.tensor_reduce(
                            mx[:], bc_v, axis=mybir.AxisListType.X, op=ALU.max
                        )
                        eq = rows.tile([128, m], f32, tag="eq", name="eq")
                        eq_v = eq[:].rearrange("p (b l) -> p b l", b=BL)
                        nc.vector.tensor_tensor(
                            eq_v, bc_v,
                            mx[:].unsqueeze(2).broadcast_to((128, BL, Lc)),
                            op=ALU.is_equal,
                        )
                        nc.vector.tensor_tensor(
                            row_view(sel_m, 0, Lc), eq_v, act_b, op=ALU.mult
                        )
                        if Lc > 1:
                            iota_b = (
                                iota128[:, :Lc]
                                .unsqueeze(1)
                                .broadcast_to((128, BL, Lc))
                            )
                            nc.vector.tensor_tensor(eq_v, eq_v, iota_b, op=ALU.mult)
                            kidx = rows.tile([128, BL], f32, tag="kidx", name="kidx")
                            nc.vector.tensor_reduce(
                                kidx[:], eq_v, axis=mybir.AxisListType.X, op=ALU.add
                            )
                            nc.vector.tensor_tensor(
                                eq_v, iota_b,
                                kidx[:].unsqueeze(2).broadcast_to((128, BL, Lc)),
                                op=ALU.is_gt,
                            )
                            rm_m = masks.tile(
                                [128, 512], dt.uint8, tag="rmm", name="rm_m"
                            )
                            nc.vector.tensor_tensor(
                                row_view(rm_m, 0, Lc), eq_v, act_b, op=ALU.mult
                            )
                    else:
                        nc.vector.tensor_copy(
                            row_view(sel_m, 0, 1),
                            act128[:, i * BL : (i + 1) * BL].unsqueeze(2),
                        )

                    # ---- in-place blend, chunk 0 first so PE restarts early
                    do_shift = i < L - 2 and Lc > 1
                    for f in range(NF):
                        for st, src in zip(hstate, hnew):
                            nc.vector.copy_predicated(
                                chunk_view(st, f, 0, Lc),
                                row_view(sel_m, 0, Lc),
                                chunk_view(src, f, 0, Lc),
                            )
                            if do_shift:
                                nc.vector.copy_predicated(
                                    chunk_view(st, f, 0, Lc),
                                    row_view(rm_m, 0, Lc),
                                    chunk_view(st, f, 1, Lc),
                                )
                    for f in range(NF):
                        nc.vector.copy_predicated(
                            chunk_view(c_t, f, 0, Lc),
                            row_view(sel_m, 0, Lc),
                            chunk_view(cn_t, f, 0, Lc),
                        )
                        if do_shift:
                            nc.vector.copy_predicated(
                                chunk_view(c_t, f, 0, Lc),
                                row_view(rm_m, 0, Lc),
                                chunk_view(c_t, f, 1, Lc),
                            )

            # ---------------- output ----------------
            with tc.tile_pool(name="fin", bufs=1) as fin:
                if precision == "fp16x2":
                    hroot = fin.tile([128, NF, BL], f32)
                    nc.vector.tensor_add(
                        hroot[:],
                        col_view(hh_t, 0, 1).squeeze(3),
                        col_view(hl_t, 0, 1).squeeze(3),
                    )
                    nc.sync.dma_start(
                        outh_d[:].rearrange("p (c b) -> p c b", b=BL), hroot[:]
                    )
                else:
                    nc.sync.dma_start(
                        outh_d[:].rearrange("p (c b) -> p c b", b=BL),
                        col_view(h_t, 0, 1).squeeze(3),
                    )
                nc.sync.dma_start(
                    outc_d[:].rearrange("p (c b) -> p c b", b=BL),
                    col_view(c_t, 0, 1).squeeze(3),
                )

    nc.compile()
    return nc


def _prep_inputs(inputs, precision=PRECISION):
    inp = np.ascontiguousarray(np.asarray(inputs["inp"], dtype=np.float32))
    length = np.asarray(inputs["length"]).astype(np.int64)
    Ww = np.asarray(inputs["Ww"], dtype=np.float32)
    bw = np.asarray(inputs["bw"], dtype=np.float32)
    Wc = np.asarray(inputs["Wc"], dtype=np.float32)
    bc = np.asarray(inputs["bc"], dtype=np.float32)
    q = np.asarray(inputs["q"], dtype=np.float32)

    wwT = np.ascontiguousarray(Ww.T)                      # [512, 1024]
    wcT = np.ascontiguousarray(Wc.T)                      # [1024, 2560]
    bw_t = np.ascontiguousarray(bw.reshape(NK, 128).T)
    bc_adj = bc.copy()
    bc_adj[H : 3 * H] += 1.0                              # +1 on both forget gates
    bc_t = np.ascontiguousarray(bc_adj.reshape(NN, 128).T)
    q_t = np.ascontiguousarray(q.reshape(NF, 128).T)
    iota_row = np.arange(64, dtype=np.float32).reshape(1, 64)
    ones_row = np.ones((1, 128), np.float32)

    shared = {
        "wwT": wwT,
        "bw_t": bw_t,
        "bc_t": bc_t,
        "q_t": q_t,
        "iota_row": iota_row,
        "ones_row": ones_row,
    }
    if precision == "fp16x2":
        wcTh = wcT.astype(np.float16)
        wcTl = (wcT - wcTh.astype(np.float32)).astype(np.float16)
        shared["wcTh"] = wcTh
        shared["wcTl"] = wcTl
    else:
        shared["wcT"] = wcT

    in_maps = []
    for c in range(NCORES):
        sl = slice(c * BL, (c + 1) * BL)
        inpT = np.ascontiguousarray(inp[sl].reshape(BL * L, W).T)  # [512, 512]
        lc = length[sl]
        act = np.zeros((1, 512), np.float32)
        for i in range(L - 1):
            act[0, i * BL : (i + 1) * BL] = (lc > i).astype(np.float32)
        in_maps.append({"inpT": inpT, "act_row": act, **shared})
    return in_maps


def _postprocess(results):
    hs, cs = [], []
    for r in results:
        oh = r["out_h"].reshape(128, NF, BL).transpose(2, 1, 0).reshape(BL, H)
        oc = r["out_c"].reshape(128, NF, BL).transpose(2, 1, 0).reshape(BL, H)
        hs.append(oh)
        cs.append(oc)
    return np.concatenate(hs, 0), np.concatenate(cs, 0)


def kernel(**inputs):
    if "nc" not in _cached:
        _cached["nc"] = _build()
    nc = _cached["nc"]
    in_maps = _prep_inputs(inputs)
    res = run_bass_kernel_spmd(nc, in_maps, list(range(NCORES)))
    return _postprocess(res.results)


def kernel_profiled(**inputs):
    """test.py helper: also returns BassKernelResults for timing/trace."""
    if "nc" not in _cached:
        _cached["nc"] = _build()
    nc = _cached["nc"]
    in_maps = _prep_inputs(inputs)
    try:
        res = run_bass_kernel_spmd(nc, in_maps, list(range(NCORES)), trace=True)
    except Exception as e:
        print("trace failed, running untraced:", e)
        res = run_bass_kernel_spmd(nc, in_maps, list(range(NCORES)))
    return _postprocess(res.results), res


# revision 9
# speedup vs baseline: 1.9772x; 1.2180x over previous
"""BinaryTreeLSTM (easy-first / Gumbel TreeLSTM, eval-mode hard argmax) on 8 TRN2
NeuronCores.

Strategy (sharding hint): data-parallel over batch. Each core runs the full
63-step depth loop for its 8 sentences, entirely SBUF-resident, feature-major
(h/c as [128 part, 4 chunks, 512 cols] tiles, column = sentence*64 + position).

Numerics: the argmax selection is brutal — min top-2 score gap over the run is
4.7e-6, and one flipped argmax rebuilds a different tree for that sentence
(absmax error ~0.8). bf16 matmuls flip 370 times; fp32(4cy/row) and an
fp16 hi/lo split (3 passes at 1cy/row, error ~1e-7) both flip zero times
(verified offline against the fixed key-0 inputs). Default: fp16x2 — the
recurrent h state is kept permanently split as (hh, hl) fp16 pairs; the column
blend is pure copies so the split survives exactly; c stays fp32 (never enters
a matmul).

Per step i (Lc = 63-i pairs, m = 8*Lc):
  PE : v[n] = sum_k [Wch_k^T hh_k + Wch_k^T hl_k + Wcl_k^T hh_k]  (20 n x 8 k x 3)
  ACT: 20 gate tiles sigmoid/tanh straight out of PSUM (bias pre-folded)
  DVE: per-chunk c_new/h_new combine (chunk-pipelined under the matmuls)
  PE : scores = q . h_new -> [1,(8,Lc)] PSUM, then K=1 ones-outer-product
       broadcast of the score row to 128 partitions (keeps masks full-width)
  DVE: per-sentence argmax -> one-hot sel mask + right-shift mask, uint8,
       written in the b*64+l column layout
  DVE: in-place blend per chunk: st[k*] <- new[k*]; st[l] <- st[l+1] (l>k*)
       for st in {hh, hl, c}
"""

import numpy as np

import concourse.bass as bass
import concourse.tile as tile
from concourse import bacc, mybir
from concourse.bass_utils import run_bass_kernel_spmd

dt = mybir.dt
AF = mybir.ActivationFunctionType
ALU = mybir.AluOpType

B, L, W, H = 64, 64, 512, 512
NCORES = 8
BL = B // NCORES          # sentences per core
K2H = 2 * H               # 1024 contraction dim
N5H = 5 * H               # 2560 output dim
NK = K2H // 128           # 8 k-chunks
NN = N5H // 128           # 20 n-tiles
NF = H // 128             # 4 feature chunks

PRECISION = "fp16x2"      # "fp32" fallback

_cached = {}


def _snake_order(length):
    """Ranks sentences by descending length; rank r -> core r%8, slot r//8."""
    return np.argsort(-np.asarray(length), kind="stable")


def _active_counts(length):
    order = _snake_order(length)
    length = np.asarray(length)
    a = np.zeros(L - 1, np.int64)
    for i in range(L - 1):
        a[i] = max(
            int((length[order[c::NCORES]] > i).sum()) for c in range(NCORES)
        )
    return tuple(int(x) for x in a)


def _build(amax, precision=PRECISION):
    nc = bacc.Bacc()
    f32 = dt.float32
    f16 = dt.float16

    inpT_d = nc.declare_dram_parameter("inpT", [W, BL * L], f32, isOutput=False)
    wwT_d = nc.declare_dram_parameter("wwT", [W, K2H], f32, isOutput=False)
    if precision == "fp16x2":
        wcTh_d = nc.declare_dram_parameter("wcTh", [K2H, N5H], f16, isOutput=False)
        wcTl_d = nc.declare_dram_parameter("wcTl", [K2H, N5H], f16, isOutput=False)
    else:
        wcT_d = nc.declare_dram_parameter("wcT", [K2H, N5H], f32, isOutput=False)
    bw_d = nc.declare_dram_parameter("bw_t", [128, NK], f32, isOutput=False)
    bc_d = nc.declare_dram_parameter("bc_t", [128, NN], f32, isOutput=False)
    q_d = nc.declare_dram_parameter("q_t", [128, NF], f32, isOutput=False)
    act_d = nc.declare_dram_parameter("act_row", [1, 512], f32, isOutput=False)
    iota_d = nc.declare_dram_parameter("iota_row", [1, 64], f32, isOutput=False)
    ones_d = nc.declare_dram_parameter("ones_row", [1, 128], f32, isOutput=False)
    outh_d = nc.declare_dram_parameter("out_h", [128, NF * BL], f32, isOutput=True)
    outc_d = nc.declare_dram_parameter("out_c", [128, NF * BL], f32, isOutput=True)

    def col_view(t, off, Lc, a=BL):
        # [128, NF, a, Lc] view of a [128, NF, 512] tile at position offset
        return t[:].rearrange("p c (b l) -> p c b l", l=64)[:, :, :a, off : off + Lc]

    def chunk_view(t, f, off, Lc, a=BL):
        # [128, a, Lc] view of chunk f of a [128, NF, 512] tile
        return t[:, f, :].rearrange("p (b l) -> p b l", l=64)[:, :a, off : off + Lc]

    def row_view(t, off, Lc, a=BL):
        # [128, a, Lc] view of a [128, 512] tile
        return t[:].rearrange("p (b l) -> p b l", l=64)[:, :a, off : off + Lc]

    with tile.TileContext(nc) as tc:
        with (
            tc.tile_pool(name="persist", bufs=1) as persist,
            tc.tile_pool(name="psum", bufs=1, space="PSUM") as psum,
        ):
            if precision == "fp16x2":
                wch_t = persist.tile([128, NK, N5H], f16)
                wcl_t = persist.tile([128, NK, N5H], f16)
                for k in range(NK):
                    nc.sync.dma_start(
                        wch_t[:, k, :],
                        wcTh_d[:].rearrange("(k p) n -> p k n", p=128)[:, k, :],
                    )
                    nc.sync.dma_start(
                        wcl_t[:, k, :],
                        wcTl_d[:].rearrange("(k p) n -> p k n", p=128)[:, k, :],
                    )
            else:
                wc_t = persist.tile([128, NK, N5H], f32)
                for k in range(NK):
                    nc.sync.dma_start(
                        wc_t[:, k, :],
                        wcT_d[:].rearrange("(k p) n -> p k n", p=128)[:, k, :],
                    )
            bc_t = persist.tile([128, NN], f32)
            nc.sync.dma_start(bc_t[:], bc_d[:])
            bw_t = persist.tile([128, NK], f32)
            nc.sync.dma_start(bw_t[:], bw_d[:])
            q_t = persist.tile([128, NF], f32)
            nc.sync.dma_start(q_t[:], q_d[:])
            act_t = persist.tile([1, 512], f32)
            nc.sync.dma_start(act_t[:], act_d[:])
            iota_t = persist.tile([1, 64], f32)
            nc.sync.dma_start(iota_t[:], iota_d[:])
            ones_t = persist.tile([1, 128], f32)
            nc.sync.dma_start(ones_t[:], ones_d[:])

            iota128 = persist.tile([128, 64], f32)
            nc.gpsimd.partition_broadcast(iota128[:], iota_t[:])
            act128 = persist.tile([128, 512], f32)
            nc.gpsimd.partition_broadcast(act128[:], act_t[:])

            # recurrent state
            if precision == "fp16x2":
                hh_t = persist.tile([128, NF, 512], f16)
                hl_t = persist.tile([128, NF, 512], f16)
                hstate = [hh_t, hl_t]
            else:
                h_t = persist.tile([128, NF, 512], f32)
                hstate = [h_t]
            c_t = persist.tile([128, NF, 512], f32)

            # ---------------- phase 0: word linear (fp32) ----------------
            with tc.tile_pool(name="ph0", bufs=1) as ph0:
                ww_t = ph0.tile([128, 4, K2H], f32)
                for k in range(4):
                    nc.sync.dma_start(
                        ww_t[:, k, :],
                        wwT_d[:].rearrange("(k p) n -> p k n", p=128)[:, k, :],
                    )
                ix_t = ph0.tile([128, 4, BL * L], f32)
                for k in range(4):
                    nc.sync.dma_start(
                        ix_t[:, k, :],
                        inpT_d[:].rearrange("(k p) m -> p k m", p=128)[:, k, :],
                    )
                for n in range(NK):
                    p0 = psum.tile([128, BL * L], f32, tag="v", bufs=6, name="p0")
                    for k in range(4):
                        nc.tensor.matmul(
                            p0[:],
                            ww_t[:, k, n * 128 : (n + 1) * 128],
                            ix_t[:, k, :],
                            start=(k == 0),
                            stop=(k == 3),
                        )
                    if n < NF:
                        if precision == "fp16x2":
                            nc.scalar.activation(
                                hh_t[:, n, :], p0[:], AF.Identity,
                                bias=bw_t[:, n : n + 1],
                            )
                            nc.vector.scalar_tensor_tensor(
                                hl_t[:, n, :], p0[:], bw_t[:, n : n + 1],
                                hh_t[:, n, :], op0=ALU.add, op1=ALU.subtract,
                            )
                        else:
                            nc.scalar.activation(
                                h_t[:, n, :], p0[:], AF.Identity,
                                bias=bw_t[:, n : n + 1],
                            )
                    else:
                        nc.scalar.activation(
                            c_t[:, n - NF, :], p0[:], AF.Identity,
                            bias=bw_t[:, n : n + 1],
                        )

            # ---------------- 63 tree steps ----------------
            with (
                tc.tile_pool(name="gates", bufs=1) as gates,
                tc.tile_pool(name="temps", bufs=1) as temps,
                tc.tile_pool(name="rows", bufs=1) as rows,
                tc.tile_pool(name="masks", bufs=1) as masks,
            ):
                for i in range(L - 1):
                    Lc = L - 1 - i
                    a = amax[i]
                    if a == 0:
                        continue
                    m = a * Lc

                    g_i = gates.tile([128, NF, m], f32, tag="g0", name="g_i")
                    g_fl = gates.tile([128, NF, m], f32, tag="g1", name="g_fl")
                    g_fr = gates.tile([128, NF, m], f32, tag="g2", name="g_fr")
                    g_u = gates.tile([128, NF, m], f32, tag="g3", name="g_u")
                    g_o = gates.tile([128, NF, m], f32, tag="g4", name="g_o")
                    gtiles = [g_i, g_fl, g_fr, g_u, g_o]
                    gfuncs = [AF.Sigmoid, AF.Sigmoid, AF.Sigmoid, AF.Tanh, AF.Sigmoid]

                    m1 = temps.tile([128, NF, m], f32, tag="m1", name="m1")
                    m2 = temps.tile([128, NF, m], f32, tag="m2", name="m2")
                    m3 = temps.tile([128, NF, m], f32, tag="m3", name="m3")
                    cn_t = temps.tile([128, NF, 512], f32, tag="cn", name="cn_t")
                    hn_t = temps.tile([128, NF, 512], f32, tag="hn", name="hn_t")
                    if precision == "fp16x2":
                        hnh_t = temps.tile([128, NF, 512], f16, tag="hnh", name="hnh_t")
                        hnl_t = temps.tile([128, NF, 512], f16, tag="hnl", name="hnl_t")
                        hnew = [hnh_t, hnl_t]
                    else:
                        hnew = [hn_t]

                    if i < L - 2:
                        ps_s = psum.tile([1, m], f32, tag="s", bufs=1, name="ps_s")

                    for f in range(NF):
                        # ---- composition matmuls for the 5 gates of chunk f
                        for g in range(5):
                            n = g * NF + f
                            vt = psum.tile([128, m], f32, tag="v", bufs=6, name="vt")
                            for k in range(NK):
                                fo = k % NF
                                off = 0 if k < NF else 1
                                if precision == "fp16x2":
                                    xh = chunk_view(hh_t, fo, off, Lc, a)
                                    xl = chunk_view(hl_t, fo, off, Lc, a)
                                    wh = wch_t[:, k, n * 128 : (n + 1) * 128]
                                    wl = wcl_t[:, k, n * 128 : (n + 1) * 128]
                                    nc.tensor.matmul(
                                        vt[:], wh, xh, start=(k == 0), stop=False
                                    )
                                    nc.tensor.matmul(
                                        vt[:], wh, xl, start=False, stop=False
                                    )
                                    nc.tensor.matmul(
                                        vt[:], wl, xh, start=False,
                                        stop=(k == NK - 1),
                                    )
                                else:
                                    nc.tensor.matmul(
                                        vt[:],
                                        wc_t[:, k, n * 128 : (n + 1) * 128],
                                        chunk_view(h_t, fo, off, Lc, a),
                                        start=(k == 0),
                                        stop=(k == NK - 1),
                                    )
                            nc.scalar.activation(
                                gtiles[g][:, f, :], vt[:], gfuncs[g],
                                bias=bc_t[:, n : n + 1],
                            )

                        # ---- combine for chunk f (overlaps later chunks' MMs)
                        cl_f = chunk_view(c_t, f, 0, Lc, a)
                        cr_f = chunk_view(c_t, f, 1, Lc, a)
                        cn_f = chunk_view(cn_t, f, 0, Lc, a)
                        hn_f = chunk_view(hn_t, f, 0, Lc, a)
                        nc.vector.tensor_mul(m1[:, f, :], g_fl[:, f, :], cl_f)
                        nc.vector.tensor_mul(m2[:, f, :], g_fr[:, f, :], cr_f)
                        nc.gpsimd.tensor_tensor(
                            m3[:, f, :], g_u[:, f, :], g_i[:, f, :], op=ALU.mult
                        )
                        nc.vector.tensor_add(m1[:, f, :], m1[:, f, :], m2[:, f, :])
                        nc.vector.tensor_add(cn_f, m1[:, f, :], m3[:, f, :])
                        nc.scalar.activation(m2[:, f, :], cn_f, AF.Tanh)
                        nc.vector.tensor_mul(hn_f, g_o[:, f, :], m2[:, f, :])
                        if i < L - 2:
                            nc.tensor.matmul(
                                ps_s[:],
                                q_t[:, f : f + 1],
                                hn_f,
                                start=(f == 0),
                                stop=(f == NF - 1),
                            )

                    if precision == "fp16x2":
                        # hi/lo split of h_new off the DVE critical path
                        nc.scalar.copy(
                            col_view(hnh_t, 0, Lc, a), col_view(hn_t, 0, Lc, a)
                        )
                        nc.gpsimd.tensor_tensor(
                            col_view(hnl_t, 0, Lc, a),
                            col_view(hn_t, 0, Lc, a),
                            col_view(hnh_t, 0, Lc, a),
                            op=ALU.subtract,
                        )

                    # ---- selection masks (b*64+l layout, uint8)
                    # every processed sentence is active (length-sorted prefix)
                    sel_m = masks.tile([128, 512], dt.uint8, tag="selm", name="sel_m")
                    if i < L - 2:
                        srow = rows.tile([1, m], f32, tag="srow", name="srow")
                        nc.vector.tensor_copy(srow[:], ps_s[:])
                        ps_bc = psum.tile([128, m], f32, tag="bc", bufs=1, name="ps_bc")
                        nc.tensor.matmul(
                            ps_bc[:], ones_t[0:1, :], srow[:], start=True, stop=True
                        )
                        bc_v = ps_bc[:].rearrange("p (b l) -> p b l", b=a)
                        act_b = (
                            act128[:, i * BL : i * BL + a]
                            .unsqueeze(2)
                            .broadcast_to((128, a, Lc))
                        )
                        mx = rows.tile([128, a], f32, tag="mx", name="mx")
                        nc.vector.tensor_reduce(
                            mx[:], bc_v, axis=mybir.AxisListType.X, op=ALU.max
                        )
                        eq = rows.tile([128, m], f32, tag="eq", name="eq")
                        eq_v = eq[:].rearrange("p (b l) -> p b l", b=a)
                        nc.vector.tensor_tensor(
                            eq_v, bc_v,
                            mx[:].unsqueeze(2).broadcast_to((128, a, Lc)),
                            op=ALU.is_equal,
                        )
                        nc.vector.tensor_tensor(
                            row_view(sel_m, 0, Lc, a), eq_v, act_b, op=ALU.mult
                        )
                        if Lc > 1:
                            iota_b = (
                                iota128[:, :Lc]
                                .unsqueeze(1)
                                .broadcast_to((128, a, Lc))
                            )
                            nc.vector.tensor_tensor(eq_v, eq_v, iota_b, op=ALU.mult)
                            kidx = rows.tile([128, a], f32, tag="kidx", name="kidx")
                            nc.vector.tensor_reduce(
                                kidx[:], eq_v, axis=mybir.AxisListType.X, op=ALU.add
                            )
                            nc.vector.tensor_tensor(
                                eq_v, iota_b,
                                kidx[:].unsqueeze(2).broadcast_to((128, a, Lc)),
                                op=ALU.is_gt,
                            )
                            rm_m = masks.tile(
                                [128, 512], dt.uint8, tag="rmm", name="rm_m"
                            )
                            nc.vector.tensor_tensor(
                                row_view(rm_m, 0, Lc, a), eq_v, act_b, op=ALU.mult
                            )
                    else:
                        nc.vector.tensor_copy(
                            row_view(sel_m, 0, 1, a),
                            act128[:, i * BL : i * BL + a].unsqueeze(2),
                        )

                    # ---- in-place blend, chunk 0 first so PE restarts early
                    do_shift = i < L - 2 and Lc > 1
                    for f in range(NF):
                        for st, srct in zip(hstate, hnew):
                            nc.vector.copy_predicated(
                                chunk_view(st, f, 0, Lc, a),
                                row_view(sel_m, 0, Lc, a),
                                chunk_view(srct, f, 0, Lc, a),
                            )
                            if do_shift:
                                nc.vector.copy_predicated(
                                    chunk_view(st, f, 0, Lc, a),
                                    row_view(rm_m, 0, Lc, a),
                                    chunk_view(st, f, 1, Lc, a),
                                )
                    for f in range(NF):
                        nc.vector.copy_predicated(
                            chunk_view(c_t, f, 0, Lc, a),
                            row_view(sel_m, 0, Lc, a),
                            chunk_view(cn_t, f, 0, Lc, a),
                        )
                        if do_shift:
                            nc.vector.copy_predicated(
                                chunk_view(c_t, f, 0, Lc, a),
                                row_view(rm_m, 0, Lc, a),
                                chunk_view(c_t, f, 1, Lc, a),
                            )

            # ---------------- output ----------------
            with tc.tile_pool(name="fin", bufs=1) as fin:
                if precision == "fp16x2":
                    hroot = fin.tile([128, NF, BL], f32)
                    nc.vector.tensor_add(
                        hroot[:],
                        col_view(hh_t, 0, 1).squeeze(3),
                        col_view(hl_t, 0, 1).squeeze(3),
                    )
                    nc.sync.dma_start(
                        outh_d[:].rearrange("p (c b) -> p c b", b=BL), hroot[:]
                    )
                else:
                    nc.sync.dma_start(
                        outh_d[:].rearrange("p (c b) -> p c b", b=BL),
                        col_view(h_t, 0, 1).squeeze(3),
                    )
                nc.sync.dma_start(
                    outc_d[:].rearrange("p (c b) -> p c b", b=BL),
                    col_view(c_t, 0, 1).squeeze(3),
                )

    nc.compile()
    return nc


def _prep_inputs(inputs, precision=PRECISION):
    inp = np.ascontiguousarray(np.asarray(inputs["inp"], dtype=np.float32))
    length = np.asarray(inputs["length"]).astype(np.int64)
    order = _snake_order(length)
    Ww = np.asarray(inputs["Ww"], dtype=np.float32)
    bw = np.asarray(inputs["bw"], dtype=np.float32)
    Wc = np.asarray(inputs["Wc"], dtype=np.float32)
    bc = np.asarray(inputs["bc"], dtype=np.float32)
    q = np.asarray(inputs["q"], dtype=np.float32)

    wwT = np.ascontiguousarray(Ww.T)                      # [512, 1024]
    wcT = np.ascontiguousarray(Wc.T)                      # [1024, 2560]
    bw_t = np.ascontiguousarray(bw.reshape(NK, 128).T)
    bc_adj = bc.copy()
    bc_adj[H : 3 * H] += 1.0                              # +1 on both forget gates
    bc_t = np.ascontiguousarray(bc_adj.reshape(NN, 128).T)
    q_t = np.ascontiguousarray(q.reshape(NF, 128).T)
    iota_row = np.arange(64, dtype=np.float32).reshape(1, 64)
    ones_row = np.ones((1, 128), np.float32)

    shared = {
        "wwT": wwT,
        "bw_t": bw_t,
        "bc_t": bc_t,
        "q_t": q_t,
        "iota_row": iota_row,
        "ones_row": ones_row,
    }
    del bw, bc, q
    if precision == "fp16x2":
        wcTh = wcT.astype(np.float16)
        wcTl = (wcT - wcTh.astype(np.float32)).astype(np.float16)
        shared["wcTh"] = wcTh
        shared["wcTl"] = wcTl
    else:
        shared["wcT"] = wcT

    in_maps = []
    for c in range(NCORES):
        idx = order[c::NCORES]                 # this core's sentences (by rank)
        inpT = np.ascontiguousarray(inp[idx].reshape(BL * L, W).T)  # [512, 512]
        lc = length[idx]
        act = np.zeros((1, 512), np.float32)
        for i in range(L - 1):
            act[0, i * BL : (i + 1) * BL] = (lc > i).astype(np.float32)
        in_maps.append({"inpT": inpT, "act_row": act, **shared})
    return in_maps


def _postprocess(results, order):
    h = np.empty((B, H), np.float32)
    c = np.empty((B, H), np.float32)
    for ci, r in enumerate(results):
        oh = r["out_h"].reshape(128, NF, BL).transpose(2, 1, 0).reshape(BL, H)
        oc = r["out_c"].reshape(128, NF, BL).transpose(2, 1, 0).reshape(BL, H)
        idx = order[ci::NCORES]
        h[idx] = oh
        c[idx] = oc
    return h, c


def _get_nc(inputs):
    amax = _active_counts(np.asarray(inputs["length"]).astype(np.int64))
    key = (PRECISION, amax)
    if key not in _cached:
        _cached[key] = _build(amax)
    return _cached[key]


def kernel(**inputs):
    nc = _get_nc(inputs)
    in_maps = _prep_inputs(inputs)
    res = run_bass_kernel_spmd(nc, in_maps, list(range(NCORES)))
    order = _snake_order(np.asarray(inputs["length"]).astype(np.int64))
    return _postprocess(res.results, order)


def kernel_profiled(**inputs):
    """test.py helper: also returns BassKernelResults for timing/trace."""
    nc = _get_nc(inputs)
    in_maps = _prep_inputs(inputs)
    try:
        res = run_bass_kernel_spmd(nc, in_maps, list(range(NCORES)), trace=True)
    except Exception as e:
        print("trace failed, running untraced:", e)
        res = run_bass_kernel_spmd(nc, in_maps, list(range(NCORES)))
    order = _snake_order(np.asarray(inputs["length"]).astype(np.int64))
    return _postprocess(res.results, order), res
